# revision 18
# baseline (speedup 1.0000x reference)
"""Trainium2 Bass kernel for nn_ContrastLoss_Disentangle.

Contract: kernel(**inputs) takes the FULL (unsharded) inputs and returns the
same structure the reference returns: (loss_label, loss_norm, loss_triple)
as float32 scalars.

Pipeline (8 NeuronCores, data-parallel):
  host:    norms (exact), normalization, categories folded into nlp rows
           (g = nlpF * cat), everything scaled x16 and cast to fp8_e4m3,
           transposed to [C*D, rows] layouts pre-swizzled for SBUF
  device1: per-core: scores via fp8 DoubleRow PE matmuls (diag-extracted
           with a mask on DVE/ACT) + a [512, 1024] block of the pm gram
           matrix (fp8 DoubleRow PE, descaled fp8 output)
  host:    BCE, stable argsort rank-select (furthest), gather+pack of the
           label-1 "hard positive" g columns
  device2: per-core: dots of the packed columns, additive -1e9 mask and
           reduce-max -> per-pose maxcur directly
  host:    triplet loss assembly

All heavy HBM traffic is fp8 (nlp rows: 2 MB/core, pose gram operands:
3 MB/core, packed hard-positives: ~1.3 MB/core) which puts both kernels
near the serialized-DMA roofline of the part.  Accuracy: fp8 x16 rounding
perturbs scores by ~0.5% absolute and reshuffles `furthest` between
near-rank poses; measured end-to-end max rel err ~8e-4 (gate 2e-2).
"""

import os
import numpy as np
import ml_dtypes

import concourse.bass as bass
import concourse.tile as tile
from concourse import bacc, mybir
from concourse.bass2jax import install_neuronx_cc_hook, partition_id_tensor, _bass_exec_p

C, NP, K, D = 8, 2048, 4, 256
NN = NP * K          # 8192
NCORES = 8
NPL = NP // NCORES   # 256 poses per core
NNL = NN // NCORES   # 1024 nlp rows per core
CD = C * D           # 2048 contraction size
KT = CD // 128       # 16 k-tiles

SC = float(os.environ.get("FP8_SCALE", "16"))
F8 = ml_dtypes.float8_e4m3
DR = os.environ.get("DR", "1") == "1"
W2 = 288             # packed label-1 columns per 128-pose half (mean 256);
                     # overflow handled exactly on the host (rare)
NEG = -1.0e9

# pm block grid: 4 row-blocks x 2 col-blocks
PM_MI, PM_NJ = 4, 2
PM_M = NP // PM_MI   # 512 rows per core block
PM_N = NP // PM_NJ   # 1024 cols per core block

_runners = {}


def _build_kernel(with_pm: bool):
    """Per-core program. Inputs (per core, pre-swizzled so partition p holds
    contraction dims k*128+p and every DMA line is contiguous):
      k1 (with_pm):
        g    [128, 2, KT, 512] fp8  nlp-side columns, hh-major; col 128*m+p
                                    maps to local nlp row 512*hh+4*p+m
        hl   [128, 2, KT, 256] fp8  pose columns, [own 256 | other 256] of
                                    the pm row-block (rows permuted on host)
        hr   [128, 2, KT, 512] fp8  pm rhs pose columns, nb-major
        mask [128, 512] bf16        mask[p, 128*m+q] = (q == p)
        outs: sc [128, 8] f32 (col = hh*4 + m);  pm8 [512, 1024] fp8 (/256)
      k2 (packed hard-positives):
        g    [128, 2, KT, W2] fp8   packed label-1 columns per half
        hl   [128, KT, 256] fp8     own pose columns
        maskn [128, 2*W2] bf16      0 where column belongs to pose p, -1e9
                                    elsewhere (incl. padding)
        outs: mx [128, 2] f32       per-pose max of masked dots
    """
    nc = bacc.Bacc("TRN2", target_bir_lowering=False, debug=False,
                   num_devices=NCORES)
    f8 = mybir.dt.float8e4
    if with_pm:
        g_in = nc.dram_tensor("g", [128, 2, KT, 512], f8,
                              kind="ExternalInput").ap()
        hl_in = nc.dram_tensor("hl", [128, 2, KT, 256], f8,
                               kind="ExternalInput").ap()
        hr_in = nc.dram_tensor("hr", [128, 2, KT, 512], f8,
                               kind="ExternalInput").ap()
        mask_in = nc.dram_tensor("mask", [128, 512], mybir.dt.bfloat16,
                                 kind="ExternalInput").ap()
        pmo = nc.dram_tensor("pm8", [PM_M, PM_N], f8,
                             kind="ExternalOutput").ap()
        sc_out = nc.dram_tensor("sc", [128, 8], mybir.dt.float32,
                                kind="ExternalOutput").ap()
    else:
        g_in = nc.dram_tensor("g", [128, 2, KT, W2], f8,
                              kind="ExternalInput").ap()
        hl_in = nc.dram_tensor("hl", [128, KT, 256], f8,
                               kind="ExternalInput").ap()
        mask_in = nc.dram_tensor("maskn", [128, 2 * W2], mybir.dt.bfloat16,
                                 kind="ExternalInput").ap()
        mx_out = nc.dram_tensor("mx", [128, 2], mybir.dt.float32,
                                kind="ExternalOutput").ap()

    with tile.TileContext(nc) as tc:
        with tc.tile_pool(name="big", bufs=1) as big, \
             tc.tile_pool(name="scr", bufs=4) as scr, \
             tc.tile_pool(name="ev", bufs=2) as ev, \
             tc.tile_pool(name="ps", bufs=4, space="PSUM") as ps:

            if with_pm:
                hl_t = big.tile([128, 2, KT, 256], f8, tag="hl")
                g_t = big.tile([128, 2, KT, 512], f8, tag="g")
                hr_t = big.tile([128, 2, KT, 512], f8, tag="hr")
                mask_t = big.tile([128, 512], mybir.dt.bfloat16, tag="mask")
                sc_t = big.tile([128, 8], mybir.dt.float32, tag="sc")
            else:
                hl_t = big.tile([128, KT, 256], f8, tag="hl")
                g_t = big.tile([128, 2, KT, W2], f8, tag="g")
                mask_t = big.tile([128, 2 * W2], mybir.dt.bfloat16,
                                  tag="mask")
                mx_t = big.tile([128, 2], mybir.dt.float32, tag="mx")

            # ---- DMA stream: critical inputs first, mask early (it gates
            # the extractions and psum-slot reuse) --------------------------
            if with_pm:
                nc.sync.dma_start(hl_t[:, 0], hl_in[:, 0])     # own poses
            else:
                nc.sync.dma_start(hl_t[:], hl_in)
            nc.sync.dma_start(mask_t[:], mask_in)
            for hh in range(2):
                nc.sync.dma_start(g_t[:, hh, 0:8], g_in[:, hh, 0:8])
                nc.sync.dma_start(g_t[:, hh, 8:16], g_in[:, hh, 8:16])
            if with_pm:
                nc.sync.dma_start(hl_t[:, 1], hl_in[:, 1])
                for nb in range(2):
                    for kq in range(4):
                        nc.sync.dma_start(
                            hr_t[:, nb, 4 * kq:4 * (kq + 1)],
                            hr_in[:, nb, 4 * kq:4 * (kq + 1)])

            def hl_slice(half, kp, col, w):
                if with_pm:
                    return hl_t[:, half, 2 * kp:2 * kp + 2, col:col + w]
                return hl_t[:, 2 * kp:2 * kp + 2, col:col + w]

            def mm(acc, half, lcol, rt, rsel, rcol, w):
                if DR:
                    for kp in range(KT // 2):
                        nc.tensor.matmul(
                            acc[:], hl_slice(half, kp, lcol, 128),
                            rt[:, rsel, 2 * kp:2 * kp + 2, rcol:rcol + w],
                            start=(kp == 0), stop=(kp == KT // 2 - 1),
                            perf_mode=mybir.MatmulPerfMode.DoubleRow)
                else:
                    for k in range(KT):
                        if with_pm:
                            lh = hl_t[:, half, k, lcol:lcol + 128]
                        else:
                            lh = hl_t[:, k, lcol:lcol + 128]
                        nc.tensor.matmul(
                            acc[:], lh, rt[:, rsel, k, rcol:rcol + w],
                            start=(k == 0), stop=(k == KT - 1))

            if with_pm:
                # scores: diag blocks of (own poses)^T @ g, quarter psums;
                # q0 extracts on DVE, q1 on DVE-mult + ACT accum-copies
                for hh in range(2):
                    for q in range(2):
                        acc_s = ps.tile([128, 512], mybir.dt.float32,
                                        tag="ps", name=f"accs{hh}{q}")
                        mm(acc_s[:, 0:256], 0, 128 * hh, g_t, hh,
                           256 * q, 256)
                        z = scr.tile([128, 256], mybir.dt.float32, tag="z",
                                     name=f"z{hh}{q}")
                        nc.vector.tensor_tensor(
                            z[:], acc_s[:, 0:256],
                            mask_t[:, 256 * q:256 * (q + 1)],
                            op=mybir.AluOpType.mult)
                        if q == 0:
                            nc.vector.tensor_reduce(
                                sc_t[:, 4 * hh:4 * hh + 2],
                                z[:].rearrange("p (m w) -> p m w", m=2),
                                axis=mybir.AxisListType.X,
                                op=mybir.AluOpType.add)
                        else:
                            zd = scr.tile([128, 128], mybir.dt.float32,
                                          tag="zd", name=f"zd{hh}")
                            for m in range(2):
                                nc.scalar.activation(
                                    zd[:], z[:, 128 * m:128 * (m + 1)],
                                    mybir.ActivationFunctionType.Copy,
                                    accum_out=sc_t[:, 4 * hh + 2 + m:
                                                   4 * hh + 3 + m])
                nc.sync.dma_start(sc_out, sc_t[:])

                # pm block: hl^T @ hr, kp-outer so the last hr chunk only
                # gates the final kp pair; evacs alternate ACT/DVE with a
                # 1/256 descale; each evac engine issues its own out-DMA
                # (wait already satisfied -> no SEQ blocking on SP)
                for nb in range(2):
                    o = ev.tile([128, 4, 512], mybir.dt.float8e4, tag="ev",
                                name=f"ev{nb}")
                    accs = [ps.tile([128, 512], mybir.dt.float32, tag="ps",
                                    name=f"acc{nb}{mb}") for mb in range(4)]
                    if DR:
                        for kp in range(KT // 2):
                            for mb in range(4):
                                nc.tensor.matmul(
                                    accs[mb][:],
                                    hl_slice(mb // 2, kp, 128 * (mb % 2), 128),
                                    hr_t[:, nb, 2 * kp:2 * kp + 2, :],
                                    start=(kp == 0), stop=(kp == KT // 2 - 1),
                                    perf_mode=mybir.MatmulPerfMode.DoubleRow)
                    else:
                        for k in range(KT):
                            for mb in range(4):
                                nc.tensor.matmul(
                                    accs[mb][:],
                                    hl_t[:, mb // 2, k,
                                         128 * (mb % 2):128 * (mb % 2) + 128],
                                    hr_t[:, nb, k, :],
                                    start=(k == 0), stop=(k == KT - 1))
                    for mb in range(4):
                        dst = (pmo[:, 512 * nb:512 * (nb + 1)]
                               .rearrange("(mb p) n -> p mb n", p=128)
                               [:, mb:mb + 1])
                        if mb % 2 == 0:
                            nc.scalar.activation(
                                o[:, mb], accs[mb][:],
                                mybir.ActivationFunctionType.Copy,
                                scale=1.0 / (SC * SC))
                            nc.scalar.dma_start(dst, o[:, mb:mb + 1])
                        else:
                            nc.vector.tensor_scalar_mul(
                                o[:, mb], accs[mb][:], 1.0 / (SC * SC))
                            nc.gpsimd.dma_start(dst, o[:, mb:mb + 1])
            else:
                # packed hard-positives: masked dots, reduce-max per pose
                for hh in range(2):
                    acc_s = ps.tile([128, W2], mybir.dt.float32, tag="ps",
                                    name=f"accm{hh}")
                    mm(acc_s, 0, 128 * hh, g_t, hh, 0, W2)
                    z = scr.tile([128, W2], mybir.dt.float32, tag="z",
                                 name=f"zm{hh}")
                    nc.vector.tensor_tensor(
                        z[:], acc_s[:], mask_t[:, W2 * hh:W2 * (hh + 1)],
                        op=mybir.AluOpType.add)
                    nc.vector.tensor_reduce(
                        mx_t[:, hh:hh + 1], z[:],
                        axis=mybir.AxisListType.X, op=mybir.AluOpType.max)
                nc.sync.dma_start(mx_out, mx_t[:])

    nc.finalize()
    return nc


def _make_runner(nc):
    """Reusable jitted SPMD runner (replicates bass2jax.run_bass_via_pjrt but
    caches the compiled executable across calls)."""
    import jax
    from jax.sharding import Mesh, PartitionSpec
    from jax.experimental.shard_map import shard_map

    install_neuronx_cc_hook()
    partition_name = nc.partition_id_tensor.name if nc.partition_id_tensor else None
    in_names, out_names, out_avals = [], [], []
    for alloc in nc.m.functions[0].allocations:
        if not isinstance(alloc, mybir.MemoryLocationSet):
            continue
        name = alloc.memorylocations[0].name
        if alloc.kind == "ExternalInput":
            if name != partition_name:
                in_names.append(name)
        elif alloc.kind == "ExternalOutput":
            out_names.append(name)
            out_avals.append(jax.core.ShapedArray(
                tuple(alloc.tensor_shape), mybir.dt.np(alloc.dtype)))
    n_params = len(in_names)
    all_in = in_names + out_names + ([partition_name] if partition_name else [])

    def _body(*args):
        operands = list(args)
        if partition_name is not None:
            operands.append(partition_id_tensor())
        outs = _bass_exec_p.bind(
            *operands, out_avals=tuple(out_avals), in_names=tuple(all_in),
            out_names=tuple(out_names), lowering_input_output_aliases=(),
            sim_require_finite=False, sim_require_nnan=False, nc=nc)
        return tuple(outs)

    devices = jax.devices()[:NCORES]
    mesh = Mesh(np.asarray(devices), ("core",))
    donate = tuple(range(n_params, n_params + len(out_names)))
    sharded = jax.jit(
        shard_map(_body, mesh=mesh,
                  in_specs=(PartitionSpec("core"),) * (n_params + len(out_names)),
                  out_specs=(PartitionSpec("core"),) * len(out_names),
                  check_rep=False),
        donate_argnums=donate, keep_unused=True)

    def run(in_maps):
        concat_in = [np.concatenate([np.asarray(m[name]) for m in in_maps], axis=0)
                     for name in in_names]
        zeros = [np.zeros((NCORES * a.shape[0], *a.shape[1:]), a.dtype)
                 for a in out_avals]
        out_arrs = sharded(*concat_in, *zeros)
        return [
            {name: np.asarray(out_arrs[i]).reshape(NCORES, *out_avals[i].shape)[c]
             for i, name in enumerate(out_names)}
            for c in range(NCORES)
        ]

    return run


def _get_runner(key):
    if key not in _runners:
        _runners[key] = _make_runner(_build_kernel(with_pm=(key == "k1")))
    return _runners[key]


def _swz(x):
    """[CD, W] (uint8/fp8) -> [128, KT, W] with partition p holding
    contraction rows k*128+p."""
    return np.ascontiguousarray(x.reshape(KT, 128, x.shape[1]).transpose(1, 0, 2))


def _swz2(x):
    """[CD, 2*W] -> [128, 2, KT, W]: like _swz but with the two column
    halves split out as a leading chunk axis (contiguous DMA chunks)."""
    w = x.shape[1] // 2
    return np.ascontiguousarray(
        x.reshape(KT, 128, 2, w).transpose(1, 2, 0, 3))


def _kernel_host_fallback(inputs):
    """Pure-numpy reference replication, used only if the index tensors do
    not have the canonical arange structure the device layout relies on."""
    nlp = np.asarray(inputs["nlp_features"], np.float32)
    pose = np.asarray(inputs["pose_features"], np.float32)
    nlab = np.asarray(inputs["nlp_label"]).astype(np.int64)
    n2p = np.asarray(inputs["nlpid2poseid"]).astype(np.int64)
    p2n = np.asarray(inputs["pose2nlpid"]).astype(np.int64)
    cat = np.asarray(inputs["categories"], np.float32)
    ri = np.asarray(inputs["rand_index"]).astype(np.int64)
    Np, Nn = pose.shape[1], nlp.shape[1]
    norm_p = np.sqrt(np.einsum("cpd,cpd->cp", pose, pose, dtype=np.float32))
    norm_n = np.sqrt(np.einsum("cnd,cnd->cn", nlp, nlp, dtype=np.float32))
    poseF = pose / norm_p[:, :, None]
    nlpF = nlp / norm_n[:, :, None]
    loss_norm = np.float32(np.float32(norm_p.mean()) + np.float32(norm_n.mean()))
    dots = np.einsum("cnd,cnd->cn", nlpF, poseF[:, n2p]).astype(np.float32)
    scores = np.einsum("cn,nc->n", dots, cat).astype(np.float32)
    p = (1.0 / (1.0 + np.exp(-scores))).astype(np.float32)
    lblf = nlab.astype(np.float32)
    loss_label = np.float32(
        np.mean(-(np.log(p) * lblf + np.log(1.0 - p) * (1.0 - lblf))))
    pf = np.ascontiguousarray(poseF.transpose(0, 2, 1).reshape(-1, Np))
    pm = (pf.T @ pf).astype(np.float32)
    ar = np.arange(Np)
    pm[ar, ar] = 1.0
    order = np.argsort(pm, axis=1, kind="stable")
    furthest = order[ar, ri]
    sg = scores[p2n]
    lg = nlab[p2n]
    maxp = np.maximum(np.max(np.where(lg == 0, sg, -np.inf), axis=1), -1.0)
    minp = np.minimum(np.min(np.where(lg == 1, sg, np.inf), axis=1), 1.0)
    nids = p2n[furthest]
    cd = np.einsum("cpkd,cpd->cpk", nlpF[:, nids], poseF)
    cur = np.einsum("cpk,pkc->pk", cd, cat[nids]).astype(np.float32)
    lcur = nlab[nids]
    maxcur = np.max(np.where(lcur == 1, cur, -np.inf), axis=1)
    maxp = np.maximum(maxp, maxcur)
    found = ~((maxp == -1.0) | (minp == 1.0))
    lt = np.where(found, maxp - minp + 2.0, 0.0).astype(np.float32)
    nf = int(np.sum(~found))
    loss_triple = (np.float32(0.0) if nf == Nn else
                   np.float32(lt.sum(dtype=np.float32) / np.float32(Nn - nf)))
    return (np.float32(loss_label), loss_norm, loss_triple)


def kernel(**inputs):
    nlp = np.ascontiguousarray(inputs["nlp_features"], np.float32)      # [C, NN, D]
    pose = np.ascontiguousarray(inputs["pose_features"], np.float32)    # [C, NP, D]
    nlab = np.asarray(inputs["nlp_label"]).astype(np.int64)
    cat = np.ascontiguousarray(inputs["categories"], np.float32)        # [NN, C]
    ri = np.asarray(inputs["rand_index"]).astype(np.int64)

    n2p = np.asarray(inputs["nlpid2poseid"]).astype(np.int64)
    p2n = np.asarray(inputs["pose2nlpid"]).astype(np.int64)
    if (not np.array_equal(n2p, np.arange(NN) // K)
            or not np.array_equal(p2n, np.arange(NN).reshape(NP, K))):
        return _kernel_host_fallback(inputs)

    # ---- host: exact norms, normalize, fold categories, fp8 x16 ---------
    norm_p = np.sqrt(np.einsum("cpd,cpd->cp", pose, pose, dtype=np.float32,
                               optimize=True)).astype(np.float32)       # [C, NP]
    norm_n = np.sqrt(np.einsum("cnd,cnd->cn", nlp, nlp, dtype=np.float32,
                               optimize=True)).astype(np.float32)       # [C, NN]
    loss_norm = np.float32(np.float32(norm_p.mean()) + np.float32(norm_n.mean()))

    poseF = pose / norm_p[:, :, None]
    hT8 = np.ascontiguousarray(
        (poseF * SC).transpose(0, 2, 1)).reshape(CD, NP).astype(F8)     # [CD, NP]

    gscale = (cat.T / norm_n) * SC                                      # [C, NN]
    g8 = (nlp * gscale[:, :, None]).astype(F8)                          # [C, NN, D]
    g8T = np.ascontiguousarray(g8.transpose(0, 2, 1)).reshape(CD, NN)   # [CD, NN]
    # (p, m) -> (m, p) within each 512-column block so the device's diag
    # mask lines up: col 512*hh + 128*m + p <- local row 512*hh + 4*p + m
    g8km = np.ascontiguousarray(
        g8T.reshape(CD, NN // 512, 128, 4).transpose(0, 1, 3, 2)
    ).reshape(CD, NN)

    mask = np.zeros((128, 512), np.float32)
    mask[np.arange(128)[:, None], 128 * np.arange(4)[None, :] + np.arange(128)[:, None]] = 1.0
    mask = mask.astype(ml_dtypes.bfloat16)

    # ---- device kernel 1 -------------------------------------------------
    # hl column order per core: own 256 poses first (so the scores matmul
    # can address them at a fixed offset), then the other half of the pm
    # row-block; pm rows are written back through rows_order.
    run1 = _get_runner("k1")
    in1 = []
    rows_order = np.empty((NCORES, PM_M), np.int64)
    for c in range(NCORES):
        i, j = c // PM_NJ, c % PM_NJ
        par = c % 2
        own = np.arange(512 * i + 256 * par, 512 * i + 256 * par + 256)
        oth = np.arange(512 * i + 256 * (1 - par), 512 * i + 256 * (1 - par) + 256)
        cols = np.concatenate([own, oth])
        rows_order[c] = cols
        in1.append({
            "g": _swz2(g8km[:, c * NNL:(c + 1) * NNL]),
            "hl": _swz2(hT8[:, cols]),
            "hr": _swz2(hT8[:, PM_N * j:PM_N * (j + 1)]),
            "mask": mask,
        })
    res1 = run1(in1)

    # ---- host: scores / BCE ---------------------------------------------
    sc_all = np.stack([r["sc"] for r in res1])                          # [8,128,8]
    scores = (sc_all.reshape(NCORES, 128, 2, 4).transpose(0, 2, 1, 3)
              .reshape(NN) / np.float32(SC * SC)).astype(np.float32)
    p = (1.0 / (1.0 + np.exp(-scores))).astype(np.float32)
    lblf = nlab.astype(np.float32)
    loss_label = np.float32(
        np.mean(-(np.log(p) * lblf + np.log(1.0 - p) * (1.0 - lblf))))

    # ---- host: furthest selection ---------------------------------------
    pm = np.empty((NP, NP), np.float32)
    for c in range(NCORES):
        j = c % PM_NJ
        pm[rows_order[c], PM_N * j:PM_N * (j + 1)] = \
            res1[c]["pm8"].astype(np.float32)
    ar = np.arange(NP)
    pm[ar, ar] = np.float32(1.0)
    order = np.argsort(pm, axis=1, kind="stable")
    furthest = order[ar, ri]                                            # [NP]

    sg = scores.reshape(NP, K)
    lg = nlab.reshape(NP, K)
    maxp = np.maximum(np.max(np.where(lg == 0, sg, -np.inf), axis=1), -1.0)
    minp = np.minimum(np.min(np.where(lg == 1, sg, np.inf), axis=1), 1.0)

    # ---- device kernel 2: packed label-1 hard-positive columns ----------
    f4 = furthest[:, None] * K + np.arange(K)                           # [NP, K]
    lab4 = nlab[f4] == 1                                                # [NP, K]
    mflat = lab4.reshape(NCORES, 2, 512)                                # (c,hh,(p,k))
    pos = np.cumsum(mflat, axis=2) - 1
    valid = mflat & (pos < W2)
    ci, hi, ei = np.nonzero(valid)
    pcols = pos[valid]
    src = f4.reshape(NCORES, 2, 512)[valid]                             # g rows
    g2u = np.zeros((CD, NCORES, 2, W2), np.uint8)
    g2u[:, ci, hi, pcols] = g8T.view(np.uint8)[:, src]
    maskn = np.full((NCORES, 128, 2, W2), NEG, np.float32)
    maskn[ci, ei // K, hi, pcols] = 0.0
    maskn = maskn.reshape(NCORES, 128, 2 * W2).astype(ml_dtypes.bfloat16)

    run2 = _get_runner("k2")
    in2 = []
    for c in range(NCORES):
        gc = g2u[:, c].reshape(CD, 2 * W2)
        in2.append({
            "g": _swz2(gc).view(F8),
            "hl": _swz(hT8[:, NPL * c:NPL * (c + 1)]),
            "maskn": maskn[c],
        })
    res2 = run2(in2)
    mx = np.stack([r["mx"] for r in res2])                              # [8,128,2]
    maxcur = np.where(mx > -1.0e8, mx / np.float32(SC * SC), -np.inf)
    maxcur = maxcur.transpose(0, 2, 1).reshape(NP)                      # (c,hh,p)

    # overflowed packed columns (> W2 label-1 entries per half): host dots
    if valid.sum() != lab4.sum():
        off = mflat & (pos >= W2)
        for c0, h0, e0 in zip(*np.nonzero(off)):
            q = c0 * NPL + h0 * 128 + e0 // K
            r = f4.reshape(NCORES, 2, 512)[c0, h0, e0]
            v = float(np.dot(g8T[:, r].astype(np.float32),
                             hT8[:, q].astype(np.float32))) / (SC * SC)
            maxcur[q] = max(maxcur[q], v)

    maxp = np.maximum(maxp, maxcur)
    found = ~((maxp == -1.0) | (minp == 1.0))
    lt = np.where(found, maxp - minp + 2.0, 0.0).astype(np.float32)
    not_find = int(np.sum(~found))
    if not_find == NN:
        loss_triple = np.float32(0.0)
    else:
        loss_triple = np.float32(lt.sum(dtype=np.float32) / np.float32(NN - not_find))

    return (np.float32(loss_label), np.float32(loss_norm), np.float32(loss_triple))


# revision 24
# speedup vs baseline: 1.0757x; 1.0757x over previous
"""Trainium2 Bass kernel for nn_ContrastLoss_Disentangle.

Contract: kernel(**inputs) takes the FULL (unsharded) inputs and returns the
same structure the reference returns: (loss_label, loss_norm, loss_triple)
as float32 scalars.

Pipeline (8 NeuronCores, data-parallel):
  host:    norms (exact), normalization, categories folded into nlp rows
           (g = nlpF * cat), everything scaled x16 and cast to fp8_e4m3,
           transposed to [C*D, rows] layouts pre-swizzled for SBUF
  device1: per-core: scores via fp8 DoubleRow PE matmuls (diag-extracted
           with a mask on DVE/ACT) + a [512, 1024] block of the pm gram
           matrix (fp8 DoubleRow PE, descaled fp8 output)
  host:    BCE, stable argsort rank-select (furthest), gather+pack of the
           label-1 "hard positive" g columns
  device2: per-core: dots of the packed columns, additive -1e9 mask and
           reduce-max -> per-pose maxcur directly
  host:    triplet loss assembly

All heavy HBM traffic is fp8 (nlp rows: 2 MB/core, pose gram operands:
3 MB/core, packed hard-positives: ~1.3 MB/core) which puts both kernels
near the serialized-DMA roofline of the part.  Accuracy: fp8 x16 rounding
perturbs scores by ~0.5% absolute and reshuffles `furthest` between
near-rank poses; measured end-to-end max rel err ~8e-4 (gate 2e-2).
"""

import os
import numpy as np
import ml_dtypes

import concourse.bass as bass
import concourse.tile as tile
from concourse import bacc, mybir
from concourse.bass2jax import install_neuronx_cc_hook, partition_id_tensor, _bass_exec_p

C, NP, K, D = 8, 2048, 4, 256
NN = NP * K          # 8192
NCORES = 8
NPL = NP // NCORES   # 256 poses per core
NNL = NN // NCORES   # 1024 nlp rows per core
CD = C * D           # 2048 contraction size
KT = CD // 128       # 16 k-tiles

SC = float(os.environ.get("FP8_SCALE", "16"))
F8 = ml_dtypes.float8_e4m3
DR = os.environ.get("DR", "1") == "1"
W2 = 288             # packed label-1 columns per 128-pose half (mean 256);
                     # overflow handled exactly on the host (rare)
NEG = -1.0e9

# pm block grid: 4 row-blocks x 2 col-blocks
PM_MI, PM_NJ = 4, 2
PM_M = NP // PM_MI   # 512 rows per core block
PM_N = NP // PM_NJ   # 1024 cols per core block

_runners = {}


def _build_kernel(with_pm: bool):
    """Per-core program. Inputs (per core, pre-swizzled so partition p holds
    contraction dims k*128+p and every DMA line is contiguous):
      k1 (with_pm):
        g    [128, 2, KT, 512] fp8  nlp-side columns, hh-major; col 128*m+p
                                    maps to local nlp row 512*hh+4*p+m
        hl   [128, 2, KT, 256] fp8  pose columns, [own 256 | other 256] of
                                    the pm row-block (rows permuted on host)
        hr   [128, 2, KT, 512] fp8  pm rhs pose columns, nb-major
        mask [128, 512] bf16        mask[p, 128*m+q] = (q == p)
        outs: sc [128, 8] f32 (col = hh*4 + m);  pm8 [512, 1024] fp8 (/256)
      k2 (packed hard-positives):
        g    [128, 2, KT, W2] fp8   packed label-1 columns per half
        hl   [128, KT, 256] fp8     own pose columns
        maskn [128, 2*W2] bf16      0 where column belongs to pose p, -1e9
                                    elsewhere (incl. padding)
        outs: mx [128, 2] f32       per-pose max of masked dots
    """
    nc = bacc.Bacc("TRN2", target_bir_lowering=False, debug=False,
                   num_devices=NCORES)
    f8 = mybir.dt.float8e4
    if with_pm:
        g_in = nc.dram_tensor("g", [128, 2, KT, 512], f8,
                              kind="ExternalInput").ap()
        hl_in = nc.dram_tensor("hl", [128, 2, KT, 256], f8,
                               kind="ExternalInput").ap()
        hr_in = nc.dram_tensor("hr", [128, KT, 512], f8,
                               kind="ExternalInput").ap()
        pma = nc.dram_tensor("pma", [512, 512], f8,
                             kind="ExternalOutput").ap()
        pmb = nc.dram_tensor("pmb", [512, 512], f8,
                             kind="ExternalOutput").ap()
        mask_in = nc.dram_tensor("mask", [128, 512], mybir.dt.bfloat16,
                                 kind="ExternalInput").ap()
        sc_out = nc.dram_tensor("sc", [128, 8], mybir.dt.float32,
                                kind="ExternalOutput").ap()
    else:
        g_in = nc.dram_tensor("g", [128, 2, KT, W2], f8,
                              kind="ExternalInput").ap()
        hl_in = nc.dram_tensor("hl", [128, KT, 256], f8,
                               kind="ExternalInput").ap()
        mask_in = nc.dram_tensor("maskn", [128, 2 * W2], mybir.dt.bfloat16,
                                 kind="ExternalInput").ap()
        mx_out = nc.dram_tensor("mx", [128, 2], mybir.dt.float32,
                                kind="ExternalOutput").ap()

    with tile.TileContext(nc) as tc:
        with tc.tile_pool(name="big", bufs=1) as big, \
             tc.tile_pool(name="scr", bufs=4) as scr, \
             tc.tile_pool(name="ev", bufs=2) as ev, \
             tc.tile_pool(name="ps", bufs=4, space="PSUM") as ps:

            if with_pm:
                hl_t = big.tile([128, 2, KT, 256], f8, tag="hl")
                g_t = big.tile([128, 2, KT, 512], f8, tag="g")
                hr_t = big.tile([128, KT, 512], f8, tag="hr")
                mask_t = big.tile([128, 512], mybir.dt.bfloat16, tag="mask")
                sc_t = big.tile([128, 8], mybir.dt.float32, tag="sc")
            else:
                hl_t = big.tile([128, KT, 256], f8, tag="hl")
                g_t = big.tile([128, 2, KT, W2], f8, tag="g")
                mask_t = big.tile([128, 2 * W2], mybir.dt.bfloat16,
                                  tag="mask")
                mx_t = big.tile([128, 2], mybir.dt.float32, tag="mx")

            # ---- DMA stream: pm operands first (pm computes under the g
            # stream), mask early (it gates extractions) --------------------
            if with_pm:
                nc.sync.dma_start(hl_t[:, 0], hl_in[:, 0])     # own poses
                nc.sync.dma_start(mask_t[:], mask_in)
                nc.sync.dma_start(hl_t[:, 1], hl_in[:, 1])
                for kq in range(4):
                    nc.sync.dma_start(hr_t[:, 4 * kq:4 * (kq + 1)],
                                      hr_in[:, 4 * kq:4 * (kq + 1)])
            else:
                nc.sync.dma_start(hl_t[:], hl_in)
                nc.sync.dma_start(mask_t[:], mask_in)
            for hh in range(2):
                nc.sync.dma_start(g_t[:, hh, 0:8], g_in[:, hh, 0:8])
                nc.sync.dma_start(g_t[:, hh, 8:16], g_in[:, hh, 8:16])

            def hl_slice(half, kp, col, w):
                if with_pm:
                    return hl_t[:, half, 2 * kp:2 * kp + 2, col:col + w]
                return hl_t[:, 2 * kp:2 * kp + 2, col:col + w]

            def mm(acc, half, lcol, rt, rsel, rcol, w):
                if DR:
                    for kp in range(KT // 2):
                        nc.tensor.matmul(
                            acc[:], hl_slice(half, kp, lcol, 128),
                            rt[:, rsel, 2 * kp:2 * kp + 2, rcol:rcol + w],
                            start=(kp == 0), stop=(kp == KT // 2 - 1),
                            perf_mode=mybir.MatmulPerfMode.DoubleRow)
                else:
                    for k in range(KT):
                        if with_pm:
                            lh = hl_t[:, half, k, lcol:lcol + 128]
                        else:
                            lh = hl_t[:, k, lcol:lcol + 128]
                        nc.tensor.matmul(
                            acc[:], lh, rt[:, rsel, k, rcol:rcol + w],
                            start=(k == 0), stop=(k == KT - 1))

            if with_pm:
                # two pm products per core, kp-outer (4 psums each, rotating
                # through one 4-slot pool): A = hl^T @ hr, B = hr^T @ hr.
                # Across the 8 cores the (A, B) blocks cover all 10 unique
                # blocks of the symmetric pm; the host mirrors.
                def pm_product(dst, lhs_kind, oname):
                    o = ev.tile([128, 4, 512], mybir.dt.float8e4, tag="ev",
                                name=oname)
                    accs = [ps.tile([128, 512], mybir.dt.float32, tag="pp",
                                    name=f"{oname}ac{mb}") for mb in range(4)]
                    if DR:
                        for kp in range(KT // 2):
                            for mb in range(4):
                                if lhs_kind == "hl":
                                    lh = hl_slice(mb // 2, kp,
                                                  128 * (mb % 2), 128)
                                else:
                                    lh = hr_t[:, 2 * kp:2 * kp + 2,
                                              128 * mb:128 * (mb + 1)]
                                nc.tensor.matmul(
                                    accs[mb][:], lh,
                                    hr_t[:, 2 * kp:2 * kp + 2, :],
                                    start=(kp == 0),
                                    stop=(kp == KT // 2 - 1),
                                    perf_mode=mybir.MatmulPerfMode.DoubleRow)
                    else:
                        for k in range(KT):
                            for mb in range(4):
                                if lhs_kind == "hl":
                                    lh = hl_t[:, mb // 2, k,
                                              128 * (mb % 2):
                                              128 * (mb % 2) + 128]
                                else:
                                    lh = hr_t[:, k, 128 * mb:128 * (mb + 1)]
                                nc.tensor.matmul(
                                    accs[mb][:], lh, hr_t[:, k, :],
                                    start=(k == 0), stop=(k == KT - 1))
                    dr = dst.rearrange("(mb p) n -> p mb n", p=128)
                    for mb in range(2):
                        nc.scalar.activation(
                            o[:, mb], accs[mb][:],
                            mybir.ActivationFunctionType.Copy,
                            scale=1.0 / (SC * SC))
                    nc.scalar.dma_start(dr[:, 0:2], o[:, 0:2])
                    for mb in range(2, 4):
                        nc.vector.tensor_scalar_mul(
                            o[:, mb], accs[mb][:], 1.0 / (SC * SC))
                    nc.gpsimd.dma_start(dr[:, 2:4], o[:, 2:4])

                pm_product(pma, "hl", "eva")
                pm_product(pmb, "hr", "evb")

                # scores: diag blocks of (own poses)^T @ g, quarter psums;
                # q0 extracts on DVE, q1 on DVE-mult + ACT accum-copies
                for hh in range(2):
                    for q in range(2):
                        acc_s = ps.tile([128, 256], mybir.dt.float32,
                                        tag="ps", name=f"accs{hh}{q}")
                        mm(acc_s, 0, 128 * hh, g_t, hh, 256 * q, 256)
                        z = scr.tile([128, 256], mybir.dt.float32, tag="z",
                                     name=f"z{hh}{q}")
                        nc.vector.tensor_tensor(
                            z[:], acc_s[:],
                            mask_t[:, 256 * q:256 * (q + 1)],
                            op=mybir.AluOpType.mult)
                        if q == 0:
                            nc.vector.tensor_reduce(
                                sc_t[:, 4 * hh:4 * hh + 2],
                                z[:].rearrange("p (m w) -> p m w", m=2),
                                axis=mybir.AxisListType.X,
                                op=mybir.AluOpType.add)
                        else:
                            zd = scr.tile([128, 128], mybir.dt.float32,
                                          tag="zd", name=f"zd{hh}")
                            for m in range(2):
                                nc.scalar.activation(
                                    zd[:], z[:, 128 * m:128 * (m + 1)],
                                    mybir.ActivationFunctionType.Copy,
                                    accum_out=sc_t[:, 4 * hh + 2 + m:
                                                   4 * hh + 3 + m])
                nc.scalar.dma_start(sc_out, sc_t[:])
            else:
                # packed hard-positives: masked dots, reduce-max per pose
                for hh in range(2):
                    acc_s = ps.tile([128, W2], mybir.dt.float32, tag="ps",
                                    name=f"accm{hh}")
                    mm(acc_s, 0, 128 * hh, g_t, hh, 0, W2)
                    z = scr.tile([128, W2], mybir.dt.float32, tag="z",
                                 name=f"zm{hh}")
                    nc.vector.tensor_tensor(
                        z[:], acc_s[:], mask_t[:, W2 * hh:W2 * (hh + 1)],
                        op=mybir.AluOpType.add)
                    nc.vector.tensor_reduce(
                        mx_t[:, hh:hh + 1], z[:],
                        axis=mybir.AxisListType.X, op=mybir.AluOpType.max)
                nc.sync.dma_start(mx_out, mx_t[:])

    nc.finalize()
    return nc


def _make_runner(nc):
    """Reusable jitted SPMD runner (replicates bass2jax.run_bass_via_pjrt but
    caches the compiled executable across calls)."""
    import jax
    from jax.sharding import Mesh, PartitionSpec
    from jax.experimental.shard_map import shard_map

    install_neuronx_cc_hook()
    partition_name = nc.partition_id_tensor.name if nc.partition_id_tensor else None
    in_names, out_names, out_avals = [], [], []
    for alloc in nc.m.functions[0].allocations:
        if not isinstance(alloc, mybir.MemoryLocationSet):
            continue
        name = alloc.memorylocations[0].name
        if alloc.kind == "ExternalInput":
            if name != partition_name:
                in_names.append(name)
        elif alloc.kind == "ExternalOutput":
            out_names.append(name)
            out_avals.append(jax.core.ShapedArray(
                tuple(alloc.tensor_shape), mybir.dt.np(alloc.dtype)))
    n_params = len(in_names)
    all_in = in_names + out_names + ([partition_name] if partition_name else [])

    def _body(*args):
        operands = list(args)
        if partition_name is not None:
            operands.append(partition_id_tensor())
        outs = _bass_exec_p.bind(
            *operands, out_avals=tuple(out_avals), in_names=tuple(all_in),
            out_names=tuple(out_names), lowering_input_output_aliases=(),
            sim_require_finite=False, sim_require_nnan=False, nc=nc)
        return tuple(outs)

    devices = jax.devices()[:NCORES]
    mesh = Mesh(np.asarray(devices), ("core",))
    donate = tuple(range(n_params, n_params + len(out_names)))
    sharded = jax.jit(
        shard_map(_body, mesh=mesh,
                  in_specs=(PartitionSpec("core"),) * (n_params + len(out_names)),
                  out_specs=(PartitionSpec("core"),) * len(out_names),
                  check_rep=False),
        donate_argnums=donate, keep_unused=True)

    def run(in_maps):
        concat_in = [np.concatenate([np.asarray(m[name]) for m in in_maps], axis=0)
                     for name in in_names]
        zeros = [np.zeros((NCORES * a.shape[0], *a.shape[1:]), a.dtype)
                 for a in out_avals]
        out_arrs = sharded(*concat_in, *zeros)
        return [
            {name: np.asarray(out_arrs[i]).reshape(NCORES, *out_avals[i].shape)[c]
             for i, name in enumerate(out_names)}
            for c in range(NCORES)
        ]

    return run


def _get_runner(key):
    if key not in _runners:
        _runners[key] = _make_runner(_build_kernel(with_pm=(key == "k1")))
    return _runners[key]


def _swz(x):
    """[CD, W] (uint8/fp8) -> [128, KT, W] with partition p holding
    contraction rows k*128+p."""
    return np.ascontiguousarray(x.reshape(KT, 128, x.shape[1]).transpose(1, 0, 2))


def _swz2(x):
    """[CD, 2*W] -> [128, 2, KT, W]: like _swz but with the two column
    halves split out as a leading chunk axis (contiguous DMA chunks)."""
    w = x.shape[1] // 2
    return np.ascontiguousarray(
        x.reshape(KT, 128, 2, w).transpose(1, 2, 0, 3))


def _kernel_host_fallback(inputs):
    """Pure-numpy reference replication, used only if the index tensors do
    not have the canonical arange structure the device layout relies on."""
    nlp = np.asarray(inputs["nlp_features"], np.float32)
    pose = np.asarray(inputs["pose_features"], np.float32)
    nlab = np.asarray(inputs["nlp_label"]).astype(np.int64)
    n2p = np.asarray(inputs["nlpid2poseid"]).astype(np.int64)
    p2n = np.asarray(inputs["pose2nlpid"]).astype(np.int64)
    cat = np.asarray(inputs["categories"], np.float32)
    ri = np.asarray(inputs["rand_index"]).astype(np.int64)
    Np, Nn = pose.shape[1], nlp.shape[1]
    norm_p = np.sqrt(np.einsum("cpd,cpd->cp", pose, pose, dtype=np.float32))
    norm_n = np.sqrt(np.einsum("cnd,cnd->cn", nlp, nlp, dtype=np.float32))
    poseF = pose / norm_p[:, :, None]
    nlpF = nlp / norm_n[:, :, None]
    loss_norm = np.float32(np.float32(norm_p.mean()) + np.float32(norm_n.mean()))
    dots = np.einsum("cnd,cnd->cn", nlpF, poseF[:, n2p]).astype(np.float32)
    scores = np.einsum("cn,nc->n", dots, cat).astype(np.float32)
    p = (1.0 / (1.0 + np.exp(-scores))).astype(np.float32)
    lblf = nlab.astype(np.float32)
    loss_label = np.float32(
        np.mean(-(np.log(p) * lblf + np.log(1.0 - p) * (1.0 - lblf))))
    pf = np.ascontiguousarray(poseF.transpose(0, 2, 1).reshape(-1, Np))
    pm = (pf.T @ pf).astype(np.float32)
    ar = np.arange(Np)
    pm[ar, ar] = 1.0
    order = np.argsort(pm, axis=1, kind="stable")
    furthest = order[ar, ri]
    sg = scores[p2n]
    lg = nlab[p2n]
    maxp = np.maximum(np.max(np.where(lg == 0, sg, -np.inf), axis=1), -1.0)
    minp = np.minimum(np.min(np.where(lg == 1, sg, np.inf), axis=1), 1.0)
    nids = p2n[furthest]
    cd = np.einsum("cpkd,cpd->cpk", nlpF[:, nids], poseF)
    cur = np.einsum("cpk,pkc->pk", cd, cat[nids]).astype(np.float32)
    lcur = nlab[nids]
    maxcur = np.max(np.where(lcur == 1, cur, -np.inf), axis=1)
    maxp = np.maximum(maxp, maxcur)
    found = ~((maxp == -1.0) | (minp == 1.0))
    lt = np.where(found, maxp - minp + 2.0, 0.0).astype(np.float32)
    nf = int(np.sum(~found))
    loss_triple = (np.float32(0.0) if nf == Nn else
                   np.float32(lt.sum(dtype=np.float32) / np.float32(Nn - nf)))
    return (np.float32(loss_label), loss_norm, loss_triple)


def kernel(**inputs):
    nlp = np.ascontiguousarray(inputs["nlp_features"], np.float32)      # [C, NN, D]
    pose = np.ascontiguousarray(inputs["pose_features"], np.float32)    # [C, NP, D]
    nlab = np.asarray(inputs["nlp_label"]).astype(np.int64)
    cat = np.ascontiguousarray(inputs["categories"], np.float32)        # [NN, C]
    ri = np.asarray(inputs["rand_index"]).astype(np.int64)

    n2p = np.asarray(inputs["nlpid2poseid"]).astype(np.int64)
    p2n = np.asarray(inputs["pose2nlpid"]).astype(np.int64)
    if (not np.array_equal(n2p, np.arange(NN) // K)
            or not np.array_equal(p2n, np.arange(NN).reshape(NP, K))):
        return _kernel_host_fallback(inputs)

    # ---- host: exact norms, normalize, fold categories, fp8 x16 ---------
    norm_p = np.sqrt(np.einsum("cpd,cpd->cp", pose, pose, dtype=np.float32,
                               optimize=True)).astype(np.float32)       # [C, NP]
    norm_n = np.sqrt(np.einsum("cnd,cnd->cn", nlp, nlp, dtype=np.float32,
                               optimize=True)).astype(np.float32)       # [C, NN]
    loss_norm = np.float32(np.float32(norm_p.mean()) + np.float32(norm_n.mean()))

    poseF = pose / norm_p[:, :, None]
    hT8 = np.ascontiguousarray(
        (poseF * SC).transpose(0, 2, 1)).reshape(CD, NP).astype(F8)     # [CD, NP]

    gscale = (cat.T / norm_n) * SC                                      # [C, NN]
    g8 = (nlp * gscale[:, :, None]).astype(F8)                          # [C, NN, D]
    g8T = np.ascontiguousarray(g8.transpose(0, 2, 1)).reshape(CD, NN)   # [CD, NN]
    # (p, m) -> (m, p) within each 512-column block so the device's diag
    # mask lines up: col 512*hh + 128*m + p <- local row 512*hh + 4*p + m
    g8km = np.ascontiguousarray(
        g8T.reshape(CD, NN // 512, 128, 4).transpose(0, 1, 3, 2)
    ).reshape(CD, NN)

    mask = np.zeros((128, 512), np.float32)
    mask[np.arange(128)[:, None], 128 * np.arange(4)[None, :] + np.arange(128)[:, None]] = 1.0
    mask = mask.astype(ml_dtypes.bfloat16)

    # ---- device kernel 1 -------------------------------------------------
    # hl column order per core: own 256 poses first (so the scores matmul
    # can address them at a fixed offset), then the other half of the pm
    # row-block; pm rows are written back through rows_order.  Each core
    # computes pma = hl^T @ hr and pmb = hr^T @ hr; across cores these
    # cover all 10 unique blocks of the symmetric pm (HRB assignment).
    HRB = [1, 2, 2, 3, 3, 0, 0, 1]
    run1 = _get_runner("k1")
    in1 = []
    rows_order = np.empty((NCORES, PM_M), np.int64)
    for c in range(NCORES):
        i = c // 2
        par = c % 2
        own = np.arange(512 * i + 256 * par, 512 * i + 256 * par + 256)
        oth = np.arange(512 * i + 256 * (1 - par), 512 * i + 256 * (1 - par) + 256)
        cols = np.concatenate([own, oth])
        rows_order[c] = cols
        in1.append({
            "g": _swz2(g8km[:, c * NNL:(c + 1) * NNL]),
            "hl": _swz2(hT8[:, cols]),
            "hr": _swz(hT8[:, 512 * HRB[c]:512 * (HRB[c] + 1)]),
            "mask": mask,
        })
    res1 = run1(in1)

    # ---- host: scores / BCE ---------------------------------------------
    sc_all = np.stack([r["sc"] for r in res1])                          # [8,128,8]
    scores = (sc_all.reshape(NCORES, 128, 2, 4).transpose(0, 2, 1, 3)
              .reshape(NN) / np.float32(SC * SC)).astype(np.float32)
    p = (1.0 / (1.0 + np.exp(-scores))).astype(np.float32)
    lblf = nlab.astype(np.float32)
    loss_label = np.float32(
        np.mean(-(np.log(p) * lblf + np.log(1.0 - p) * (1.0 - lblf))))

    # ---- host: furthest selection (mirror the symmetric blocks) ----------
    pm = np.empty((NP, NP), np.float32)
    for c in range(NCORES):
        hrc = np.arange(512 * HRB[c], 512 * (HRB[c] + 1))
        A = res1[c]["pma"].astype(np.float32)
        pm[np.ix_(rows_order[c], hrc)] = A
        pm[np.ix_(hrc, rows_order[c])] = A.T
        pm[512 * HRB[c]:512 * (HRB[c] + 1),
           512 * HRB[c]:512 * (HRB[c] + 1)] = \
            res1[c]["pmb"].astype(np.float32)
    ar = np.arange(NP)
    pm[ar, ar] = np.float32(1.0)
    order = np.argsort(pm, axis=1, kind="stable")
    furthest = order[ar, ri]                                            # [NP]

    sg = scores.reshape(NP, K)
    lg = nlab.reshape(NP, K)
    maxp = np.maximum(np.max(np.where(lg == 0, sg, -np.inf), axis=1), -1.0)
    minp = np.minimum(np.min(np.where(lg == 1, sg, np.inf), axis=1), 1.0)

    # ---- device kernel 2: packed label-1 hard-positive columns ----------
    f4 = furthest[:, None] * K + np.arange(K)                           # [NP, K]
    lab4 = nlab[f4] == 1                                                # [NP, K]
    mflat = lab4.reshape(NCORES, 2, 512)                                # (c,hh,(p,k))
    pos = np.cumsum(mflat, axis=2) - 1
    valid = mflat & (pos < W2)
    ci, hi, ei = np.nonzero(valid)
    pcols = pos[valid]
    src = f4.reshape(NCORES, 2, 512)[valid]                             # g rows
    g2u = np.zeros((CD, NCORES, 2, W2), np.uint8)
    g2u[:, ci, hi, pcols] = g8T.view(np.uint8)[:, src]
    maskn = np.full((NCORES, 128, 2, W2), NEG, np.float32)
    maskn[ci, ei // K, hi, pcols] = 0.0
    maskn = maskn.reshape(NCORES, 128, 2 * W2).astype(ml_dtypes.bfloat16)

    run2 = _get_runner("k2")
    in2 = []
    for c in range(NCORES):
        gc = g2u[:, c].reshape(CD, 2 * W2)
        in2.append({
            "g": _swz2(gc).view(F8),
            "hl": _swz(hT8[:, NPL * c:NPL * (c + 1)]),
            "maskn": maskn[c],
        })
    res2 = run2(in2)
    mx = np.stack([r["mx"] for r in res2])                              # [8,128,2]
    maxcur = np.where(mx > -1.0e8, mx / np.float32(SC * SC), -np.inf)
    maxcur = maxcur.transpose(0, 2, 1).reshape(NP)                      # (c,hh,p)

    # overflowed packed columns (> W2 label-1 entries per half): host dots
    if valid.sum() != lab4.sum():
        off = mflat & (pos >= W2)
        for c0, h0, e0 in zip(*np.nonzero(off)):
            q = c0 * NPL + h0 * 128 + e0 // K
            r = f4.reshape(NCORES, 2, 512)[c0, h0, e0]
            v = float(np.dot(g8T[:, r].astype(np.float32),
                             hT8[:, q].astype(np.float32))) / (SC * SC)
            maxcur[q] = max(maxcur[q], v)

    maxp = np.maximum(maxp, maxcur)
    found = ~((maxp == -1.0) | (minp == 1.0))
    lt = np.where(found, maxp - minp + 2.0, 0.0).astype(np.float32)
    not_find = int(np.sum(~found))
    if not_find == NN:
        loss_triple = np.float32(0.0)
    else:
        loss_triple = np.float32(lt.sum(dtype=np.float32) / np.float32(NN - not_find))

    return (np.float32(loss_label), np.float32(loss_norm), np.float32(loss_triple))
